# revision 1
# baseline (speedup 1.0000x reference)
"""Expert-parallel MoE MLP (ExpertMLP) Bass kernel for 8 Trainium2 NeuronCores.

Problem: x[32,4096,256] @ w_fc[32,256,1024] -> gelu(erf) -> @ w_proj[32,1024,256].

Sharding: expert-parallel. Each of the 8 cores gets 4 experts (slices of the
leading axis of every tensor); no cross-core communication. Inside a core, per
expert e:

  1. x[e] ([4096,256], capacity-major) is transposed on the PE (identity
     matmul, 128x128 blocks) into xT [d, c] so the d-contraction of the first
     matmul lies on the partition axis.
  2. MM1: hT[h_tile, c_chunk] += w_fc_tile.T @ xT_chunk - w_fc's natural
     [d, h] layout is the stationary operand, so it needs no transpose.
  3. GELU (exact erf form) runs on the ACT engine as the PSUM->SBUF eviction.
  4. MM2 uses hT slices as the *stationary* operand and w_proj's natural
     [h, d] layout as the moving operand: out[c_sub, d] += hT_slice.T @
     w_proj_tile. The result lands directly in [capacity, d] orientation, so
     no output transpose is needed.

All matmul operands are float32r (e8m11, 1 PE cycle/row at N>=256 vs 4 for
fp32); producers (DVE copies / ACT gelu) write f32r tiles, which performs the
required rounding. PSUM accumulation stays fp32.
"""

import numpy as np
from contextlib import ExitStack

import bass_rust as _br
import concourse.bass as bass
import concourse.tile as tile
from concourse import mybir
from concourse.bass_utils import run_bass_kernel_spmd
from concourse.masks import make_identity

E, CAP, D, H = 32, 4096, 256, 1024
N_CORES = 8
E_PER = E // N_CORES  # 4 experts per core
P = 128
F32 = mybir.dt.float32
F32R = mybir.dt.float32r

KD = D // P        # 2 k-tiles in MM1's contraction
KH = H // P        # 8 k-tiles in MM2's contraction
NC_CHUNK = 512     # capacity chunk processed per MM1/MM2 round
N_CHUNKS = CAP // NC_CHUNK
H_TILES = H // P
C_TILES = CAP // P


def _fix_waits(nc):
    """walrus here accepts only one sync wait per instruction; hoist excess
    waits onto standalone EventSemaphore instructions inserted before the
    offender (same engine => same sequencer order)."""
    for fn in nc.m.functions:
        for bb in fn.blocks:
            new = []
            changed = False
            for inst in bb.instructions:
                si = inst.sync_info
                if si is not None and len(si.on_wait) > 1:
                    waits = list(si.on_wait)
                    for w in waits[:-1]:
                        ev = mybir.InstEventSemaphore(
                            name=nc.get_next_instruction_name()
                        )
                        ev.engine = inst.engine
                        ev.sync_info = _br.SyncInfo(on_wait=[w], on_update=[])
                        nc.register_instruction(ev)
                        new.append(ev)
                    inst.sync_info = _br.SyncInfo(
                        on_wait=waits[-1:], on_update=list(si.on_update)
                    )
                    changed = True
                new.append(inst)
            if changed:
                bb.instructions = new


def _build():
    nc = bass.Bass(trn_type="TRN2", target_bir_lowering=False, debug=False)
    x = nc.dram_tensor("x", [E_PER, CAP, D], F32, kind="ExternalInput").ap()
    w_fc = nc.dram_tensor("w_fc", [E_PER, D, H], F32, kind="ExternalInput").ap()
    w_proj = nc.dram_tensor("w_proj", [E_PER, H, D], F32, kind="ExternalInput").ap()
    out = nc.dram_tensor("out", [E_PER, CAP, D], F32, kind="ExternalOutput").ap()

    with tile.TileContext(nc) as tc, ExitStack() as ctx:
        const = ctx.enter_context(tc.tile_pool(name="const", bufs=1))
        identity = const.tile([P, P], F32)
        make_identity(nc, identity)

        xload = ctx.enter_context(tc.tile_pool(name="xload", bufs=4))
        xtp = ctx.enter_context(tc.tile_pool(name="xtp", bufs=2))
        wload = ctx.enter_context(tc.tile_pool(name="wload", bufs=2))
        wfc_p = ctx.enter_context(tc.tile_pool(name="wfc", bufs=2))
        wproj_p = ctx.enter_context(tc.tile_pool(name="wproj", bufs=2))
        ht_p = ctx.enter_context(tc.tile_pool(name="ht", bufs=16))
        out_p = ctx.enter_context(tc.tile_pool(name="outp", bufs=3))
        ps_t = ctx.enter_context(tc.tile_pool(name="ps_t", bufs=2, space="PSUM"))
        ps_h = ctx.enter_context(tc.tile_pool(name="ps_h", bufs=3, space="PSUM"))
        ps_o = ctx.enter_context(tc.tile_pool(name="ps_o", bufs=2, space="PSUM"))

        for e in range(E_PER):
            # ---- weights: DMA f32 then round to f32r on DVE ----
            wfc_raw = wload.tile([P, KD, H], F32, tag="wl")
            nc.sync.dma_start(wfc_raw[:], w_fc[e].rearrange("(k p) h -> p k h", p=P))
            wfc = wfc_p.tile([P, KD, H], F32R, tag="wfc")
            nc.vector.tensor_copy(wfc[:], wfc_raw[:])

            wproj_raw = wload.tile([P, KH, D], F32, tag="wl")
            nc.sync.dma_start(
                wproj_raw[:], w_proj[e].rearrange("(k p) d -> p k d", p=P)
            )
            wproj = wproj_p.tile([P, KH, D], F32R, tag="wproj")
            nc.vector.tensor_copy(wproj[:], wproj_raw[:])

            # ---- xT via PE transpose; DVE eviction rounds to f32r ----
            xt = xtp.tile([P, KD, CAP], F32R, tag="xt")
            for ci in range(C_TILES):
                xl = xload.tile([P, D], F32, tag="xl")
                nc.sync.dma_start(xl[:], x[e, ci * P:(ci + 1) * P, :])
                for k in range(KD):
                    pst = ps_t.tile([P, P], F32, tag="pst")
                    nc.tensor.transpose(pst[:], xl[:, k * P:(k + 1) * P], identity[:])
                    nc.vector.tensor_copy(xt[:, k, ci * P:(ci + 1) * P], pst[:])

            # ---- MM1 -> GELU -> MM2 per capacity chunk ----
            for nci in range(N_CHUNKS):
                csl = slice(nci * NC_CHUNK, (nci + 1) * NC_CHUNK)
                ht_tiles = []
                for hi in range(H_TILES):
                    psh = ps_h.tile([P, NC_CHUNK], F32, tag="psh")
                    for k in range(KD):
                        nc.tensor.matmul(
                            psh[:],
                            wfc[:, k, hi * P:(hi + 1) * P],
                            xt[:, k, csl],
                            start=(k == 0),
                            stop=(k == KD - 1),
                        )
                    ht = ht_p.tile([P, NC_CHUNK], F32R, tag="ht")
                    nc.scalar.activation(
                        ht[:], psh[:], mybir.ActivationFunctionType.Gelu
                    )
                    ht_tiles.append(ht)

                ob = out_p.tile([P, NC_CHUNK // P, D], F32, tag="ob")
                for s in range(NC_CHUNK // P):
                    pso = ps_o.tile([P, D], F32, tag="pso")
                    for k in range(KH):
                        nc.tensor.matmul(
                            pso[:],
                            ht_tiles[k][:, s * P:(s + 1) * P],
                            wproj[:, k, :],
                            start=(k == 0),
                            stop=(k == KH - 1),
                        )
                    nc.vector.tensor_copy(ob[:, s, :], pso[:])
                nc.sync.dma_start(
                    out[e, csl, :].rearrange("(s p) d -> p s d", p=P), ob[:]
                )

    _fix_waits(nc)
    return nc


_CACHE = {}


def _get_nc():
    if "nc" not in _CACHE:
        _CACHE["nc"] = _build()
    return _CACHE["nc"]


def kernel(x, w_fc, w_proj, trace=False):
    assert x.shape == (E, CAP, D) and w_fc.shape == (E, D, H)
    assert w_proj.shape == (E, H, D)
    nc = _get_nc()
    x = np.ascontiguousarray(x, dtype=np.float32)
    w_fc = np.ascontiguousarray(w_fc, dtype=np.float32)
    w_proj = np.ascontiguousarray(w_proj, dtype=np.float32)
    in_maps = [
        {
            "x": x[i * E_PER:(i + 1) * E_PER],
            "w_fc": w_fc[i * E_PER:(i + 1) * E_PER],
            "w_proj": w_proj[i * E_PER:(i + 1) * E_PER],
        }
        for i in range(N_CORES)
    ]
    res = run_bass_kernel_spmd(nc, in_maps, list(range(N_CORES)), trace=trace)
    out = np.concatenate([r["out"] for r in res.results], axis=0)
    if trace:
        kernel.last_results = res
    return out


# revision 3
# speedup vs baseline: 1.0233x; 1.0233x over previous
"""Expert-parallel MoE MLP (ExpertMLP) Bass kernel for 8 Trainium2 NeuronCores.

Problem: x[32,4096,256] @ w_fc[32,256,1024] -> gelu(erf) -> @ w_proj[32,1024,256].

Sharding: expert-parallel. Each of the 8 cores gets 4 experts (slices of the
leading axis of every tensor); no cross-core communication. Inside a core, per
expert e:

  1. x[e] ([4096,256], capacity-major) is transposed on the PE (identity
     matmul, 128x128 blocks) into xT [d, c] so the d-contraction of the first
     matmul lies on the partition axis.
  2. MM1: hT[h_tile, c_chunk] += w_fc_tile.T @ xT_chunk - w_fc's natural
     [d, h] layout is the stationary operand, so it needs no transpose.
  3. GELU (exact erf form) runs on the ACT engine as the PSUM->SBUF eviction.
  4. MM2 uses hT slices as the *stationary* operand and w_proj's natural
     [h, d] layout as the moving operand: out[c_sub, d] += hT_slice.T @
     w_proj_tile. The result lands directly in [capacity, d] orientation, so
     no output transpose is needed.

All matmul operands are float32r (e8m11, 1 PE cycle/row at N>=256 vs 4 for
fp32); producers (DVE copies / ACT gelu) write f32r tiles, which performs the
required rounding. PSUM accumulation stays fp32.
"""

import numpy as np
from contextlib import ExitStack

import bass_rust as _br
import concourse.bass as bass
import concourse.tile as tile
from concourse import mybir
from concourse.bass_utils import run_bass_kernel_spmd
from concourse.masks import make_identity

E, CAP, D, H = 32, 4096, 256, 1024
N_CORES = 8
E_PER = E // N_CORES  # 4 experts per core
P = 128
F32 = mybir.dt.float32
F32R = mybir.dt.float32r
BF16 = mybir.dt.bfloat16

KD = D // P        # 2 k-tiles in MM1's contraction
KH = H // P        # 8 k-tiles in MM2's contraction
NC_CHUNK = 512     # capacity chunk processed per MM1/MM2 round
N_CHUNKS = CAP // NC_CHUNK
H_TILES = H // P
C_TILES = CAP // P


def _fix_waits(nc):
    """walrus here accepts only one sync wait per instruction; hoist excess
    waits onto standalone EventSemaphore instructions inserted before the
    offender (same engine => same sequencer order)."""
    for fn in nc.m.functions:
        for bb in fn.blocks:
            new = []
            changed = False
            for inst in bb.instructions:
                si = inst.sync_info
                if si is not None and len(si.on_wait) > 1:
                    waits = list(si.on_wait)
                    for w in waits[:-1]:
                        ev = mybir.InstEventSemaphore(
                            name=nc.get_next_instruction_name()
                        )
                        ev.engine = inst.engine
                        ev.sync_info = _br.SyncInfo(on_wait=[w], on_update=[])
                        nc.register_instruction(ev)
                        new.append(ev)
                    inst.sync_info = _br.SyncInfo(
                        on_wait=waits[-1:], on_update=list(si.on_update)
                    )
                    changed = True
                new.append(inst)
            if changed:
                bb.instructions = new


def _build():
    nc = bass.Bass(trn_type="TRN2", target_bir_lowering=False, debug=False)
    x = nc.dram_tensor("x", [E_PER, CAP, D], F32, kind="ExternalInput").ap()
    w_fc = nc.dram_tensor("w_fc", [E_PER, D, H], F32, kind="ExternalInput").ap()
    w_proj = nc.dram_tensor("w_proj", [E_PER, H, D], F32, kind="ExternalInput").ap()
    out = nc.dram_tensor("out", [E_PER, CAP, D], F32, kind="ExternalOutput").ap()

    with tile.TileContext(nc) as tc, ExitStack() as ctx:
        const = ctx.enter_context(tc.tile_pool(name="const", bufs=1))
        identity = const.tile([P, P], F32)
        make_identity(nc, identity)

        xload = ctx.enter_context(tc.tile_pool(name="xload", bufs=4))
        xtp = ctx.enter_context(tc.tile_pool(name="xtp", bufs=2))
        wload = ctx.enter_context(tc.tile_pool(name="wload", bufs=2))
        wfc_p = ctx.enter_context(tc.tile_pool(name="wfc", bufs=2))
        wproj_p = ctx.enter_context(tc.tile_pool(name="wproj", bufs=2))
        ht_p = ctx.enter_context(tc.tile_pool(name="ht", bufs=8))
        out_p = ctx.enter_context(tc.tile_pool(name="outp", bufs=3))
        ps_t = ctx.enter_context(tc.tile_pool(name="ps_t", bufs=2, space="PSUM"))
        ps_h = ctx.enter_context(tc.tile_pool(name="ps_h", bufs=2, space="PSUM"))
        ps_o = ctx.enter_context(tc.tile_pool(name="ps_o", bufs=2, space="PSUM"))

        HPACK = 2          # h_tiles packed per PSUM tile / GELU call
        for e in range(E_PER):
            # ---- weights: DMA f32; round w_fc to f32r, w_proj to bf16 ----
            wfc_raw = wload.tile([P, KD, H], F32, tag="wl")
            nc.sync.dma_start(wfc_raw[:], w_fc[e].rearrange("(k p) h -> p k h", p=P))
            wfc = wfc_p.tile([P, KD, H], F32R, tag="wfc")
            nc.vector.tensor_copy(wfc[:], wfc_raw[:])

            wproj_raw = wload.tile([P, KH, D], F32, tag="wl")
            nc.sync.dma_start(
                wproj_raw[:], w_proj[e].rearrange("(k p) d -> p k d", p=P)
            )
            wproj = wproj_p.tile([P, KH, D], BF16, tag="wproj")
            nc.vector.tensor_copy(wproj[:], wproj_raw[:])

            # ---- xT via PE transpose; DVE eviction rounds to f32r ----
            with nc.named_scope(f"xpose{e}"):
                xt = xtp.tile([P, KD, CAP], F32R, tag="xt")
                for ci in range(C_TILES):
                    xl = xload.tile([P, D], F32, tag="xl")
                    nc.sync.dma_start(xl[:], x[e, ci * P:(ci + 1) * P, :])
                    for k in range(KD):
                        pst = ps_t.tile([P, P], F32, tag="pst")
                        nc.tensor.transpose(
                            pst[:], xl[:, k * P:(k + 1) * P], identity[:]
                        )
                        nc.vector.tensor_copy(xt[:, k, ci * P:(ci + 1) * P], pst[:])

            # ---- MM1 -> GELU -> MM2 per capacity chunk ----
            # MM1 accumulates HPACK h_tiles into one multi-bank PSUM tile so
            # GELU evicts in wider (cheaper) ACTIVATE calls; hT is written in
            # bf16 so MM2's per-matmul weight loads run at 2-byte FWL speed.
            for nci in range(N_CHUNKS):
                csl = slice(nci * NC_CHUNK, (nci + 1) * NC_CHUNK)
                ht_tiles = []  # HPACK-wide bf16 tiles
                for hp in range(H_TILES // HPACK):
                    psh = ps_h.tile([P, HPACK, NC_CHUNK], F32, tag="psh")
                    for j in range(HPACK):
                        hi = hp * HPACK + j
                        for k in range(KD):
                            nc.tensor.matmul(
                                psh[:, j, :],
                                wfc[:, k, hi * P:(hi + 1) * P],
                                xt[:, k, csl],
                                start=(k == 0),
                                stop=(k == KD - 1),
                            )
                    ht = ht_p.tile([P, HPACK, NC_CHUNK], BF16, tag="ht")
                    nc.scalar.activation(
                        ht[:], psh[:], mybir.ActivationFunctionType.Gelu
                    )
                    ht_tiles.append(ht)

                ob = out_p.tile([P, NC_CHUNK // P, D], F32, tag="ob")
                for s in range(NC_CHUNK // P):
                    pso = ps_o.tile([P, D], F32, tag="pso")
                    for k in range(KH):
                        nc.tensor.matmul(
                            pso[:],
                            ht_tiles[k // HPACK][:, k % HPACK, s * P:(s + 1) * P],
                            wproj[:, k, :],
                            start=(k == 0),
                            stop=(k == KH - 1),
                        )
                    nc.vector.tensor_copy(ob[:, s, :], pso[:])
                nc.sync.dma_start(
                    out[e, csl, :].rearrange("(s p) d -> p s d", p=P), ob[:]
                )

    _fix_waits(nc)
    return nc


_CACHE = {}


def _get_nc():
    if "nc" not in _CACHE:
        _CACHE["nc"] = _build()
    return _CACHE["nc"]


def kernel(x, w_fc, w_proj, trace=False):
    assert x.shape == (E, CAP, D) and w_fc.shape == (E, D, H)
    assert w_proj.shape == (E, H, D)
    nc = _get_nc()
    x = np.ascontiguousarray(x, dtype=np.float32)
    w_fc = np.ascontiguousarray(w_fc, dtype=np.float32)
    w_proj = np.ascontiguousarray(w_proj, dtype=np.float32)
    in_maps = [
        {
            "x": x[i * E_PER:(i + 1) * E_PER],
            "w_fc": w_fc[i * E_PER:(i + 1) * E_PER],
            "w_proj": w_proj[i * E_PER:(i + 1) * E_PER],
        }
        for i in range(N_CORES)
    ]
    res = run_bass_kernel_spmd(nc, in_maps, list(range(N_CORES)), trace=trace)
    out = np.concatenate([r["out"] for r in res.results], axis=0)
    if trace:
        kernel.last_results = res
    return out
